# revision 34
# baseline (speedup 1.0000x reference)
"""Trainium2 Bass kernel for nn_GCNN_87668872446200 (v2).

Two GCNConv+pool protein branches + two masif conv branches + dense head,
distributed over 8 NeuronCores as 4 dest-quarters x 2 feature-slices.

Optimizations vs the original baseline (1409us -> ~638us):
  - fp8(e4m3) x/W with DoubleRow matmuls for xw = x@W (W prescaled x64,
    folded back on the PSUM->fp8 copy); xw stored fp8 so the edge gather
    moves half the bytes; scatter S-matmuls are fp8 and DoubleRow-paired
    (the balanced schedule makes every block exactly 8 chunks).
  - self-loops removed from the gather stream (dma_gather is Q7
    descriptor-emission-bound, ~8.6us/1024 rows); handled densely via a
    per-core xT column permutation that puts the core's dest-quarter in
    xw rows [0, HALF), read back as a slab and folded in with a DVE
    dinv^2 scale.
  - nodes greedily rebalanced across all 4*NBLK dest blocks by in-degree
    so each block needs the minimal 8 chunks (160 total, no padding).
  - branch-2's Tensor-bound xw phase is interleaved (front-loaded) into
    branch-1's GpSimd-bound gather phase so the in-order engine queues
    overlap; deep gat/sld/idx pools keep the gather stream continuous.
  - the big exposed AllReduce (272KB) is replaced by per-core partials:
    pooled @ W_pf slices and the masif logit contribution (masif enters
    the output linearly through W_out) -> one 16.4KB bf16 AllReduce.

All 8 cores run ONE identical program; per-core variation is in input
data (weight slices, rotated xT, gather indices, S/Mpool, dinv^2, masks).
"""
import numpy as np

# ---------------------------------------------------------------- constants
N_CORES = 8
N_FSLICE = 2      # feature slices (F // N_FSLICE features per core)
N_DPART = 4       # destination-node partitions
P = 128
BLK = 128         # dest nodes per scatter block (S width)
GRP = 8           # chunks per gather/S group (1024 idxs per dma_gather)

# problem sizes (hardcoded per spec)
N_NODES, N_EDGES, F_DIM, B_GRAPHS, L_MAS, C_MAS = 10000, 80000, 1024, 32, 800, 16

MM_FP8 = True       # x/W fp8 e4m3 + DoubleRow matmuls
XW_FP8 = True       # xw storage + gather in fp8
SM_FP8 = True       # S matrix fp8 (else bf16)
W_SCALE = 64.0      # W prescale when MM_FP8 (folded back on PSUM copy)


class _Cfg:
    def __init__(self, n=N_NODES, e=N_EDGES, f=F_DIM, b=B_GRAPHS,
                 l=L_MAS, c=C_MAS):
        assert f % 512 == 0 and b == 32 and l % 80 == 0 and c % 2 == 0
        self.N, self.E, self.F, self.B, self.L, self.C = n, e, f, b, l, c
        self.NPAD = ((n + 511) // 512) * 512
        while (self.NPAD // N_DPART) % BLK or (self.NPAD % 512):
            self.NPAD += 512
        self.HALF = self.NPAD // N_DPART       # nodes per dest partition
        self.NBLK = self.HALF // BLK           # blocks per dest partition
        self.NB2 = self.HALF // P              # slab column chunks
        self.FSL = f // N_FSLICE               # features per core slice
        self.KC = f // P                       # k-chunks of contraction
        self.KCS = self.FSL // P               # k-chunks of the core's slice
        self.NT = self.NPAD // 512             # xw row tiles
        self.GPB = b // N_CORES                # graphs per core for masif
        self.LW = l // 80                      # avg-pool window (10)
        self.LB = 8                            # l-blocks for masif layout
        self.LBS = l // self.LB                # l-block size (100)
        assert self.LBS % self.LW == 0
        self.WPB = self.LBS // self.LW         # windows per l-block (10)


# ---------------------------------------------------------------- host prep
def _edge_plan(cfg, edge_index):
    """Per-quarter chunked scatter plans (self-loops excluded; they go
    through the dense slab path). Nodes within each quarter are reassigned
    to dest blocks by greedy in-degree balancing so every block needs the
    same (minimal) number of 128-edge chunks; the resulting per-quarter
    node permutation is applied to the core's xT columns so its own
    quarter is xw rows [0, HALF) in block order."""
    row = np.asarray(edge_index[0]).astype(np.int64)
    col = np.asarray(edge_index[1]).astype(np.int64)
    deg = (np.bincount(col, minlength=cfg.N) + 1).astype(np.float64)  # +self
    dinv = 1.0 / np.sqrt(deg)
    norm = (dinv[row] * dinv[col]).astype(np.float32)
    deg_in = np.bincount(col, minlength=cfg.NPAD)  # loop-free in-degree

    # global greedy balance: assign ALL nodes (desc in-degree) to the
    # least-loaded of 4*NBLK dest blocks with free capacity; quarter hf
    # owns blocks [hf*NBLK, (hf+1)*NBLK). This equalizes edge counts
    # across blocks AND quarters so every block needs the same minimal
    # number of 128-edge chunks.
    nblk_g = N_DPART * cfg.NBLK
    order = np.argsort(-deg_in, kind='stable')
    loads = np.zeros(nblk_g, np.int64)
    fill = np.zeros(nblk_g, np.int64)
    block_of = np.zeros(cfg.NPAD, np.int64)
    for q in order:
        open_ = np.flatnonzero(fill < BLK)
        j = open_[np.argmin(loads[open_])]
        block_of[q] = j
        loads[j] += deg_in[q]
        fill[j] += 1
    # perm_global[r] = node id at global row r (block-major)
    perm_global = np.argsort(block_of * cfg.NPAD + np.arange(cfg.NPAD),
                             kind='stable')
    pos_global = np.zeros(cfg.NPAD, np.int64)
    pos_global[perm_global] = np.arange(cfg.NPAD)

    quarters = []
    counts = np.zeros((N_DPART, cfg.NBLK), np.int64)
    perms = []
    for hf in range(N_DPART):
        lo = hf * cfg.HALF
        perms.append(perm_global[lo:lo + cfg.HALF])
        sel = (pos_global[col] >= lo) & (pos_global[col] < lo + cfg.HALF)
        r, c, w = row[sel], col[sel], norm[sel]
        cq = pos_global[c] - lo             # dest position in slab order
        order = np.argsort(cq, kind='stable')
        r, cq, w = r[order], cq[order], w[order]
        blk = cq // BLK
        starts = np.searchsorted(blk, np.arange(cfg.NBLK), side='left')
        ends = np.searchsorted(blk, np.arange(cfg.NBLK), side='right')
        counts[hf] = np.maximum((ends - starts + 127) // 128, 1)
        quarters.append((r, cq, w, starts, ends))

    kj = counts.max(0)                       # shared chunks per block
    c_total = int(kj.sum())
    c_pad = ((c_total + GRP - 1) // GRP) * GRP
    kj[-1] += c_pad - c_total                # tail dummies on last block

    # core-frame source rows (per quarter) + A/B split: chunks whose
    # sources all fall in xw rows [0, T) can be gathered once the first
    # half of the xw matmul has been written. Scheduling all A-chunks
    # first lets the gather stream start ~half an xw phase earlier at no
    # extra chunk cost (kA + kB == kj exactly).
    T = cfg.NPAD // 2
    rr_all, pos_all, perm_full_all = [], [], []
    count_a = np.zeros((N_DPART, cfg.NBLK), np.int64)
    for hf in range(N_DPART):
        r, cq, w, starts, ends = quarters[hf]
        perm_full = np.roll(perm_global, -hf * cfg.HALF)
        pos = np.zeros(cfg.NPAD, np.int64)
        pos[perm_full] = np.arange(cfg.NPAD)
        perm_full_all.append(perm_full)
        rr = pos[r]                          # source row in core frame
        rr_all.append(rr)
        for j in range(cfg.NBLK):
            count_a[hf, j] = int((rr[starts[j]:ends[j]] < T).sum())

    # A/B early-gather split disabled: Tile's DRAM RAW deps are
    # tensor-coarse, so sliced sources never unblock early and the extra
    # pass only added overhead (measured +43us).
    ka = np.zeros(cfg.NBLK, np.int64)
    kb = kj - ka

    # sched entries: (block, start, stop, is_b_pass)
    sched = []
    for j in range(cfg.NBLK):
        for k in range(ka[j]):
            sched.append((j, k == 0, k == ka[j] - 1, 0))
    for j in range(cfg.NBLK):
        for k in range(kb[j]):
            sched.append((j, k == 0, k == kb[j] - 1, 1))
    assert len(sched) == c_pad
    n_ga = int(ka.sum()) // GRP              # A-pure gather groups

    srcs_all, s_all = [], []
    for hf in range(N_DPART):
        r, cq, w, starts, ends = quarters[hf]
        rr = rr_all[hf]
        # per-block edge order: A-edges (src row < T) first
        seq = np.empty(len(rr), np.int64)
        ptr = 0
        bstart = np.zeros(cfg.NBLK, np.int64)
        for j in range(cfg.NBLK):
            e_idx = np.arange(starts[j], ends[j])
            amask = rr[e_idx] < T
            aidx = e_idx[amask]
            bidx = e_idx[~amask]
            # exactly ka[j]*128 A-slots; overflow A-edges go to the B part
            na = ka[j] * P
            ordered = np.concatenate([aidx[:na], aidx[na:], bidx])
            bstart[j] = ptr
            seq[ptr:ptr + len(ordered)] = ordered
            ptr += len(ordered)
        srcs = np.zeros((c_pad, P), np.int16)
        smat = np.zeros((c_pad, P, BLK), np.float32)
        used = np.zeros(cfg.NBLK, np.int64)
        for i, (j, st, sp, pb) in enumerate(sched):
            cs = bstart[j] + used[j]
            n_j = ends[j] - starts[j]
            n = min(P, n_j - used[j])
            if n > 0:
                e = seq[cs:cs + n]
                srcs[i, :n] = rr[e]
                smat[i, np.arange(n), cq[e] - j * BLK] = w[e]
                used[j] += n
        srcs_all.append(srcs)
        s_all.append(smat)
    dinv2 = (dinv * dinv).astype(np.float32)
    return (sched, c_pad, n_ga, list(ka), srcs_all, s_all, dinv2, perms,
            perm_full_all)


def _wrap_idxs(srcs):
    """[C, 128] int16 -> wrapped [128, C*8] (idx j at [j%16 + 16*rep, j//16])."""
    flat = srcs.reshape(-1)
    w = flat.reshape(-1, 16).T                # [16, C*8]
    return np.tile(w, (8, 1)).astype(np.int16)


def _group_s(smat, dt):
    """[C, 128, 64] -> [C//GRP, 128, GRP*64] grouped for contiguous loads."""
    c = smat.shape[0]
    g = smat.reshape(c // GRP, GRP, P, BLK).transpose(0, 2, 1, 3)
    return np.ascontiguousarray(g.reshape(c // GRP, P, GRP * BLK)).astype(dt)


def _mpool(cfg, batch, perm_q, dt):
    """[HALF, B] matrix folding 1/cnt (slab row r = node perm_q[r]),
    zero rows for pad nodes."""
    batch = np.asarray(batch).astype(np.int64)
    cnt = np.bincount(batch, minlength=cfg.B).astype(np.float64)
    cinv = 1.0 / np.maximum(cnt, 1.0)
    m = np.zeros((cfg.HALF, cfg.B), np.float32)
    rows = np.flatnonzero(perm_q < cfg.N)
    nodes = perm_q[rows]
    m[rows, batch[nodes]] = cinv[batch[nodes]].astype(np.float32)
    return m.astype(dt)


def _preprocess(inputs, cfg, dts):
    """Build shared program meta + per-core input maps (numpy host work)."""
    mm_dt, xw_dt, sm_dt = dts['mm'], dts['xw'], dts['sm']
    import ml_dtypes
    bf16 = ml_dtypes.bfloat16
    meta = {}
    shared = {}

    xt_rot = {}   # (br, hf) -> column-permuted xT in mm_dt
    dinv2 = {}
    perms = {}
    for br in (1, 2):
        x = np.asarray(inputs[f'pro{br}_x'], np.float32)
        xt = np.zeros((cfg.F, cfg.NPAD), mm_dt)
        xt[:, :cfg.N] = x.T.astype(mm_dt)
        (sched, c_pad, n_ga, ka, srcs, smat, d2, perm_q,
         perm_full) = _edge_plan(cfg, inputs[f'pro{br}_edge_index'])
        for hf in range(N_DPART):
            xt_rot[(br, hf)] = np.ascontiguousarray(xt[:, perm_full[hf]])
        meta[f'sched{br}'] = sched
        meta[f'cpad{br}'] = c_pad
        meta[f'nga{br}'] = n_ga
        meta[f'ka{br}'] = ka
        shared[f'_srcs{br}'] = srcs
        shared[f'_smat{br}'] = smat
        dinv2[br] = d2
        perms[br] = perm_q

    def colv(v, n):
        return np.asarray(v, np.float32).reshape(n, 1)
    shared['W_fc1'] = np.asarray(inputs['W_fc1'], np.float32)
    shared['W_fc2'] = np.asarray(inputs['W_fc2'], np.float32)
    wo = np.asarray(inputs['W_out'], np.float32)
    shared['W_out_xc'] = wo[0:64]
    shared['W_out_mas'] = wo[64:192]
    shared['b_fc1'] = colv(inputs['b_fc1'], 256)
    shared['b_fc2'] = colv(inputs['b_fc2'], 64)
    shared['b_out'] = colv(inputs['b_out'], 1)
    shared['b_pf1'] = colv(inputs['b_pf1'], 128)
    shared['b_pf2'] = colv(inputs['b_pf2'], 128)
    for m in (1, 2):
        shared[f'W_m{m}'] = (np.asarray(inputs[f'W_m{m}'], np.float32)
                             / (2.0 * cfg.LW)).reshape(8, 10, 64)
        shared[f'b_m{m}'] = colv(inputs[f'b_m{m}'], 64)
        for sf, pre in (('s', 'cs'), ('f', 'cf')):
            w = float(np.asarray(inputs[f'{pre}{m}_w'])[0])
            b = float(np.asarray(inputs[f'{pre}{m}_b'])[0])
            shared[f'scale_{sf}{m}'] = np.full((32, 1), w / cfg.C, np.float32)
            shared[f'bias_{sf}{m}'] = np.full((32, 1), b, np.float32)

    wsc = W_SCALE if MM_FP8 else 1.0
    in_maps = []
    for core in range(N_CORES):
        fs, hf = core % N_FSLICE, core // N_FSLICE
        f_lo = fs * cfg.FSL
        m = {k: v for k, v in shared.items() if not k.startswith('_')}
        for br in (1, 2):
            W = np.asarray(inputs[f'W_g{br}'], np.float32)[:, f_lo:f_lo + cfg.FSL]
            m[f'Wg{br}'] = np.ascontiguousarray(
                (W * wsc).reshape(cfg.KC, P, cfg.FSL)).astype(mm_dt)
            bia = np.asarray(inputs[f'b_g{br}'], np.float32)[f_lo:f_lo + cfg.FSL]
            m[f'bg{br}'] = np.tile(bia[None, :], (BLK, 1)).astype(bf16)
            m[f'xT{br}'] = xt_rot[(br, hf)]
            m[f'idx{br}'] = _wrap_idxs(shared[f'_srcs{br}'][hf])
            m[f'smat{br}'] = _group_s(shared[f'_smat{br}'][hf], sm_dt)
            pq = perms[br][hf]
            m[f'mpool{br}'] = _mpool(cfg, inputs[f'pro{br}_batch'], pq, bf16)
            # dinv^2 for the slab: [P, NB2], slab row c*128+p = node
            # pq[c*128+p] (zero for pad nodes)
            d2f = np.zeros((cfg.HALF,), np.float32)
            valid = pq < cfg.N
            d2f[valid] = dinv2[br][pq[valid]]
            m[f'dinv2{br}'] = np.ascontiguousarray(
                d2f.reshape(cfg.NB2, P).T)
            m[f'Wpf{br}'] = np.ascontiguousarray(
                np.asarray(inputs[f'W_pf{br}'],
                           np.float32)[f_lo:f_lo + cfg.FSL])
        gsel = slice(core * cfg.GPB, (core + 1) * cfg.GPB)
        for mi, names in ((1, ('mas1_straight', 'mas1_flipped')),
                          (2, ('mas2_straight', 'mas2_flipped'))):
            m[f'mas{mi}s'] = np.ascontiguousarray(
                np.asarray(inputs[names[0]], np.float32)[gsel])
            m[f'mas{mi}f'] = np.ascontiguousarray(
                np.asarray(inputs[names[1]], np.float32)[gsel])
        mk = np.zeros((P, cfg.B), np.float32)
        mk[:, core * cfg.GPB:(core + 1) * cfg.GPB] = 1.0
        m['gmask'] = mk
        in_maps.append(m)
    return meta, in_maps


# ---------------------------------------------------------------- program
def _build(cfg, meta, dts):
    import concourse.bass as bass
    import concourse.bacc as bacc
    import concourse.mybir as mybir
    import concourse.tile as tile
    from concourse.masks import make_identity

    dt = mybir.dt
    mm_dt = dt.from_np(np.dtype(dts['mm']))
    xw_dt = dt.from_np(np.dtype(dts['xw']))
    sm_dt = dt.from_np(np.dtype(dts['sm']))
    f32 = dt.float32
    bf16 = dt.bfloat16
    AF = mybir.ActivationFunctionType
    OP = mybir.AluOpType
    DR = mybir.MatmulPerfMode.DoubleRow

    nc = bacc.Bacc("TRN2", target_bir_lowering=False, debug=False,
                   enable_asserts=False, num_devices=N_CORES)

    def din(name, shape, d):
        return nc.dram_tensor(name, list(shape), d, kind="ExternalInput")

    xT = {br: din(f'xT{br}', (cfg.F, cfg.NPAD), mm_dt) for br in (1, 2)}
    Wg = {br: din(f'Wg{br}', (cfg.KC, P, cfg.FSL), mm_dt) for br in (1, 2)}
    bg = {br: din(f'bg{br}', (BLK, cfg.FSL), bf16) for br in (1, 2)}
    idx = {br: din(f'idx{br}', (P, meta[f'cpad{br}'] * 8), dt.int16)
           for br in (1, 2)}
    smat = {br: din(f'smat{br}', (meta[f'cpad{br}'] // GRP, P, GRP * BLK),
                    sm_dt) for br in (1, 2)}
    mpool = {br: din(f'mpool{br}', (cfg.HALF, cfg.B), bf16) for br in (1, 2)}
    dinv2 = {br: din(f'dinv2{br}', (P, cfg.NB2), f32) for br in (1, 2)}
    wpf = {br: din(f'Wpf{br}', (cfg.FSL, P), f32) for br in (1, 2)}
    b_pf = {br: din(f'b_pf{br}', (P, 1), f32) for br in (1, 2)}
    gmask = din('gmask', (P, cfg.B), f32)
    mas = {(mi, sf): din(f'mas{mi}{sf}', (cfg.GPB, cfg.C, cfg.L), f32)
           for mi in (1, 2) for sf in 'sf'}
    w_fc1 = din('W_fc1', (256, 256), f32)
    w_fc2 = din('W_fc2', (256, 64), f32)
    b_fc1 = din('b_fc1', (256, 1), f32)
    b_fc2 = din('b_fc2', (64, 1), f32)
    w_out_xc = din('W_out_xc', (64, 1), f32)
    w_out_mas = din('W_out_mas', (128, 1), f32)
    b_out = din('b_out', (1, 1), f32)
    w_m = {mi: din(f'W_m{mi}', (8, 10, 64), f32) for mi in (1, 2)}
    b_m = {mi: din(f'b_m{mi}', (64, 1), f32) for mi in (1, 2)}
    msc = {(mi, sf, kind): din(f'{kind}_{sf}{mi}', (32, 1), f32)
           for mi in (1, 2) for sf in 'sf' for kind in ('scale', 'bias')}

    out_t = nc.dram_tensor('out', [1, cfg.B], f32, kind="ExternalOutput")

    with tile.TileContext(nc) as tc:
        with tc.tile_pool(name="const", bufs=1) as cst, \
             tc.tile_pool(name="xt", bufs=6) as xtp, \
             tc.tile_pool(name="xwps", bufs=2, space="PSUM") as xwps, \
             tc.tile_pool(name="xwsb", bufs=3) as xwsb, \
             tc.tile_pool(name="gat", bufs=12) as gatp, \
             tc.tile_pool(name="sld", bufs=6) as sldp, \
             tc.tile_pool(name="idxp", bufs=6) as idxp, \
             tc.tile_pool(name="slab", bufs=2) as slabp, \
             tc.tile_pool(name="hA", bufs=2) as hap, \
             tc.tile_pool(name="blkps", bufs=2, space="PSUM") as blkps, \
             tc.tile_pool(name="hpool", bufs=4) as hp, \
             tc.tile_pool(name="poolps", bufs=2, space="PSUM") as poolps, \
             tc.tile_pool(name="small", bufs=2) as smp, \
             tc.tile_pool(name="smallps", bufs=1, space="PSUM") as smps, \
             tc.tile_pool(name="dram", bufs=2, space="DRAM") as drp:

            def load(pool, src_ap, shape, d, name=None):
                t = pool.tile(list(shape), d, tag=name)
                nc.sync.dma_start(out=t[:], in_=src_ap)
                return t

            # ---- hot consts first so PE can start early
            wg_sb = {br: load(cst, Wg[br].ap().transpose([1, 0, 2]),
                              (P, cfg.KC, cfg.FSL), mm_dt, f'wg{br}')
                     for br in (1, 2)}
            bg_sb = {br: load(cst, bg[br][:, :], (BLK, cfg.FSL), bf16,
                              f'bg{br}') for br in (1, 2)}
            mp_sb = {br: load(cst,
                              mpool[br].ap().rearrange(
                                  "(j d) g -> j d g", d=BLK).transpose([1, 0, 2]),
                              (BLK, cfg.NBLK, cfg.B), bf16, f'mp{br}')
                     for br in (1, 2)}
            d2_sb = {br: load(cst, dinv2[br][:, :], (P, cfg.NB2), f32,
                              f'd2{br}') for br in (1, 2)}
            idx_sb = {br: load(cst, idx[br][:, :],
                               (P, meta[f'cpad{br}'] * 8), dt.int16,
                               f'idx{br}') for br in (1, 2)}
            xw_dram = {br: drp.tile([cfg.NPAD, cfg.FSL], xw_dt, tag='xwdram',
                                    name=f'xwdram{br}')
                       for br in (1, 2)}

            # ---- per-phase emitters -------------------------------------
            def emit_xt_load(br, nt):
                t = xtp.tile([P, cfg.KC, 512], mm_dt, tag='xt')
                nc.sync.dma_start(
                    out=t[:],
                    in_=xT[br].ap()[:, nt * 512:(nt + 1) * 512].rearrange(
                        "(k p) n -> p k n", p=P))
                return t

            def emit_xw_compute(br, nt, xt_t):
                for sub in range(4):
                    ps = xwps.tile([P, cfg.FSL], f32, space="PSUM", tag='xwps')
                    if MM_FP8:
                        for kp in range(cfg.KC // 2):
                            nc.tensor.matmul(
                                ps[:],
                                lhsT=xt_t[:, 2 * kp:2 * kp + 2,
                                          sub * P:(sub + 1) * P],
                                rhs=wg_sb[br][:, 2 * kp:2 * kp + 2, :],
                                start=(kp == 0), stop=(kp == cfg.KC // 2 - 1),
                                perf_mode=DR)
                    else:
                        for k in range(cfg.KC):
                            nc.tensor.matmul(
                                ps[:],
                                lhsT=xt_t[:, k, sub * P:(sub + 1) * P],
                                rhs=wg_sb[br][:, k, :],
                                start=(k == 0), stop=(k == cfg.KC - 1))
                    xw_t = xwsb.tile([P, cfg.FSL], xw_dt, tag='xwsb')
                    nc.vector.tensor_scalar_mul(
                        out=xw_t[:], in0=ps[:],
                        scalar1=1.0 / W_SCALE if MM_FP8 else 1.0)
                    nc.sync.dma_start(
                        out=xw_dram[br][(nt * 4 + sub) * P:
                                        (nt * 4 + sub + 1) * P, :],
                        in_=xw_t[:])

            slab_sb = {}
            pool_ps = {}
            ha_sb = {}

            def emit_slab_load(br):
                slab_sb[br] = load(
                    slabp,
                    xw_dram[br][0:cfg.HALF, :].rearrange(
                        "(c p) f -> p c f", p=P),
                    (P, cfg.NB2, cfg.FSL), xw_dt, 'slab')

            def finish_block(br, j, blk_ps):
                c, hlo = (j * BLK) // P, (j * BLK) % P
                sl = hp.tile([BLK, cfg.FSL], bf16, tag='hself')
                nc.vector.tensor_scalar_mul(
                    out=sl[:], in0=slab_sb[br][hlo:hlo + BLK, c, :],
                    scalar1=d2_sb[br][hlo:hlo + BLK, c:c + 1])
                h_t = hp.tile([BLK, cfg.FSL], bf16, tag='h')
                nc.vector.tensor_add(out=h_t[:], in0=blk_ps[:],
                                     in1=bg_sb[br][:, :])
                nc.vector.tensor_add(out=h_t[:], in0=h_t[:], in1=sl[:])
                if meta[f'ka{br}'][j] > 0:
                    nc.vector.tensor_add(out=h_t[:], in0=h_t[:],
                                         in1=ha_sb[br][:, j, :])
                nc.scalar.activation(h_t[:], h_t[:], AF.Lrelu, alpha=0.01)
                nc.tensor.matmul(
                    pool_ps[br][:], lhsT=mp_sb[br][:, j, :], rhs=h_t[:],
                    start=(j == 0), stop=(j == cfg.NBLK - 1))

            gstate = {}

            def emit_group(br, g, split=False):
                if br not in pool_ps:
                    pool_ps[br] = poolps.tile([cfg.B, cfg.FSL], f32,
                                              space="PSUM", tag='poolps',
                                              name=f'poolps{br}')
                    if any(meta[f'ka{br}']):
                        ha_sb[br] = hap.tile([P, cfg.NBLK, cfg.FSL], xw_dt,
                                             tag='ha', name=f'ha{br}')
                    emit_slab_load(br)
                    gstate[br] = {'ci': 0, 'blk': None}
                st8 = gstate[br]
                sched = meta[f'sched{br}']
                # A-pure groups only reference xw rows [0, T): a sliced
                # source lets them start as soon as the low half is written
                in_src = (xw_dram[br][0:cfg.NPAD // 2, :]
                          if g < meta[f'nga{br}'] else xw_dram[br][:, :])
                hsz = GRP // 2 if split else GRP
                halves = []
                for hh in range(GRP // hsz):
                    ga_t = gatp.tile([P, hsz, cfg.FSL], xw_dt, tag='gat',
                                     name='gat')
                    lo = g * GRP * 8 + hh * hsz * 8
                    nc.gpsimd.dma_gather(
                        out_ap=ga_t[:],
                        in_ap=in_src,
                        idxs_ap=idx_sb[br][:, lo:lo + hsz * 8],
                        num_idxs=hsz * P, num_idxs_reg=hsz * P,
                        elem_size=cfg.FSL, single_packet=False)
                    halves.append(ga_t)
                s_t = sldp.tile([P, GRP * BLK], sm_dt, tag='sld')
                nc.sync.dma_start(out=s_t[:], in_=smat[br][g, :, :])
                s_c = s_t[:].rearrange("p (g b) -> p g b", b=BLK)
                i = 0
                while i < GRP:
                    j, st, sp, pb = sched[st8['ci']]
                    if st:
                        st8['blk'] = blkps.tile([BLK, cfg.FSL], f32,
                                                space="PSUM", tag='blkps',
                                                name='blkps')
                    ga = halves[i // hsz]
                    il = i % hsz
                    # pair consecutive same-block fp8 chunks into one
                    # DoubleRow matmul (contraction 256)
                    pair = (SM_FP8 and XW_FP8 and il + 1 < hsz
                            and sched[st8['ci'] + 1][0] == j
                            and not sched[st8['ci'] + 1][1])
                    if pair:
                        j2, st2, sp2, pb2 = sched[st8['ci'] + 1]
                        nc.tensor.matmul(
                            st8['blk'][:],
                            lhsT=s_c[:, i:i + 2, :],
                            rhs=ga[:, il:il + 2, :],
                            start=st, stop=sp2, perf_mode=DR)
                        sp, pb, j = sp2, pb2, j2
                        st8['ci'] += 2
                        i += 2
                    else:
                        nc.tensor.matmul(
                            st8['blk'][:],
                            lhsT=s_c[:, i, :],
                            rhs=ga[:, il, :],
                            start=st, stop=sp)
                        st8['ci'] += 1
                        i += 1
                    if sp:
                        if pb:
                            finish_block(br, j, st8['blk'])
                        else:
                            nc.scalar.activation(
                                ha_sb[br][:, j, :], st8['blk'][:],
                                AF.Identity)

            def emit_xpre(br):
                """pooled [B, FSL] -> x_pre partial [P, B] (this core's
                fslice contraction with W_pf); returns SBUF tile."""
                pooled_sb = smp.tile([cfg.B, cfg.FSL], f32, tag=f'pooled{br}')
                nc.scalar.activation(pooled_sb[:], pool_ps[br][:], AF.Identity)
                pfm = smp.tile([P, cfg.KCS, cfg.B], f32, tag=f'pfm{br}')
                for kk in range(cfg.KCS):
                    tps = smps.tile([P, cfg.B], f32, space="PSUM", tag='sps')
                    nc.tensor.transpose(
                        out=tps[:],
                        in_=pooled_sb[:, kk * P:(kk + 1) * P],
                        identity=id32[:])
                    nc.scalar.activation(pfm[:, kk, :], tps[:], AF.Identity)
                xps = smps.tile([P, cfg.B], f32, space="PSUM", tag='spsacc')
                for kk in range(cfg.KCS):
                    nc.tensor.matmul(xps[:], lhsT=wpf_sb[br][:, kk, :],
                                     rhs=pfm[:, kk, :],
                                     start=(kk == 0), stop=(kk == cfg.KCS - 1))
                xpre = smp.tile([P, cfg.B], bf16, tag=f'xpre{br}')
                nc.scalar.activation(xpre[:], xps[:], AF.Identity)
                return xpre

            # ---- masif (both branches) -> two [64, B] f32 tiles
            masif_asm = {1: cst.tile([64, cfg.B], f32, tag='masasm1',
                                     name='masasm1'),
                         2: cst.tile([64, cfg.B], f32, tag='masasm2',
                                     name='masasm2')}

            def emit_masif(mi):
                frag = None
                for sf in 'sf':
                    src = mas[(mi, sf)]
                    t = smp.tile([32, cfg.C, cfg.LBS], f32, tag='masload')
                    for lb in range(cfg.LB):
                        nc.sync.dma_start(
                            out=t[lb * cfg.GPB:(lb + 1) * cfg.GPB],
                            in_=src.ap()[:, :, lb * cfg.LBS:(lb + 1) * cfg.LBS])
                    red = smp.tile([32, cfg.LBS], f32, tag='masred')
                    nc.vector.tensor_reduce(
                        out=red[:], in_=t[:].transpose([0, 2, 1]),
                        axis=mybir.AxisListType.X, op=OP.add)
                    act = smp.tile([32, cfg.LBS], f32, tag='masact')
                    nc.scalar.activation(
                        act[:], red[:], AF.Relu,
                        bias=msc_sb[(mi, sf, 'bias')][:, 0:1],
                        scale=msc_sb[(mi, sf, 'scale')][:, 0:1])
                    ws = smp.tile([32, cfg.WPB], f32, tag='masws')
                    nc.vector.tensor_reduce(
                        out=ws[:],
                        in_=act[:].rearrange("p (w l) -> p w l", l=cfg.LW),
                        axis=mybir.AxisListType.X, op=OP.add)
                    if frag is None:
                        frag = ws
                    else:
                        frag2 = smp.tile([32, cfg.WPB], f32, tag='masfrag')
                        nc.vector.tensor_add(out=frag2[:], in0=frag[:], in1=ws[:])
                        frag = frag2
                ps_t = smps.tile([cfg.WPB, 32], f32, space="PSUM", tag='sps')
                nc.tensor.transpose(out=ps_t[:], in_=frag[:], identity=id32[:])
                fragT = smp.tile([cfg.WPB, 32], f32, tag='masfragT')
                nc.scalar.activation(fragT[:], ps_t[:], AF.Identity)
                fragTc = fragT[:].rearrange("k (lb g) -> k lb g", g=cfg.GPB)
                m_ps = smps.tile([64, cfg.GPB], f32, space="PSUM", tag='spsacc')
                for lb in range(cfg.LB):
                    nc.tensor.matmul(
                        m_ps[:], lhsT=wm_sb[mi][:, lb, :], rhs=fragTc[:, lb, :],
                        start=(lb == 0), stop=(lb == cfg.LB - 1))
                m_fm = smp.tile([64, cfg.GPB], f32, tag='masfm')
                nc.scalar.activation(m_fm[:], m_ps[:], AF.Identity,
                                     bias=bm_sb[mi][:, 0:1])
                nc.vector.tensor_tensor(
                    out=masif_asm[mi][:].rearrange(
                        "p (s g) -> p s g", g=cfg.GPB),
                    in0=m_fm[:, None, :].to_broadcast(
                        [64, N_CORES, cfg.GPB]),
                    in1=gmask_sb[0:64, :].rearrange(
                        "p (s g) -> p s g", g=cfg.GPB),
                    op=OP.mult)

            # ---- branch 1 xw phase (first on every queue so PE starts hot)
            for nt in range(cfg.NT):
                emit_xw_compute(1, nt, emit_xt_load(1, nt))

            # preload first branch-2 xt tiles so the interleave starts hot
            xt2_pre = {nt: emit_xt_load(2, nt) for nt in range(2)}  # noqa

            # cold consts (head weights, masif consts) after xw1's xt loads
            gmask_sb = load(cst, gmask[:, :], (P, cfg.B), f32, 'gmask')
            id32 = cst.tile([32, 32], f32, tag='id32')
            make_identity(nc, id32[:])

            wpf_sb = {br: load(cst, wpf[br].ap().rearrange(
                "(k p) m -> p k m", p=P), (P, cfg.KCS, P), f32, f'wpf{br}')
                for br in (1, 2)}
            bpf_sb = {br: load(cst, b_pf[br][:, :], (P, 1), f32, f'bpf{br}')
                      for br in (1, 2)}
            wfc1_sb = load(cst, w_fc1.ap().rearrange(
                "(a p) m -> a p m", p=P).transpose([1, 0, 2]),
                           (P, 2, 256), f32, 'wfc1')
            wfc2_sb = load(cst, w_fc2.ap().rearrange(
                "(a p) m -> a p m", p=P).transpose([1, 0, 2]),
                           (P, 2, 64), f32, 'wfc2')
            bfc1_sb = load(cst, b_fc1.ap().rearrange(
                "(a p) m -> a p m", p=P).transpose([1, 0, 2]),
                           (P, 2, 1), f32, 'bfc1')
            bfc2_sb = load(cst, b_fc2[:, :], (64, 1), f32, 'bfc2')
            woutxc_sb = load(cst, w_out_xc[:, :], (64, 1), f32, 'woutxc')
            woutmas_sb = load(cst, w_out_mas.ap().rearrange(
                "(a p) m -> p a m", p=64), (64, 2, 1), f32, 'woutmas')
            bout_sb = load(cst, b_out[:, :], (1, 1), f32, 'bout')
            wm_sb = {mi: load(cst, w_m[mi].ap().transpose([1, 0, 2]),
                              (10, 8, 64), f32, f'wm{mi}') for mi in (1, 2)}
            bm_sb = {mi: load(cst, b_m[mi][:, :], (64, 1), f32, f'bm{mi}')
                     for mi in (1, 2)}
            msc_sb = {k: load(cst, v[:, :], (32, 1), f32, f'msc{k}')
                      for k, v in msc.items()}

            # masif is cheap and only needed by AR1 -> emit after xw1 so its
            # input DMAs don't delay the xt loads at startup
            emit_masif(1)
            emit_masif(2)

            # ---- interleave: branch-1 gather/scatter with branch-2 xw.
            # Front-load xw2 (3 primed + 2 per group) so its last store
            # lands well before the branch-1 gathers finish -- gather2_0
            # waits on it and any PE lag there stalls the gather stream.
            ngrp1 = meta['cpad1'] // GRP
            ngrp2 = meta['cpad2'] // GRP
            nt2 = 0
            for _ in range(3):
                xt_t = xt2_pre.pop(nt2, None) or emit_xt_load(2, nt2)
                emit_xw_compute(2, nt2, xt_t)
                nt2 += 1
            for g in range(ngrp1):
                take = 2 if g < 5 else 1
                for _ in range(take):
                    if nt2 < cfg.NT:
                        emit_xw_compute(2, nt2, emit_xt_load(2, nt2))
                        nt2 += 1
                emit_group(1, g)
            while nt2 < cfg.NT:
                emit_xw_compute(2, nt2, emit_xt_load(2, nt2))
                nt2 += 1

            # ---- branch-1 partial head (PE work only; the collective is a
            # single AR at the very end because InstCollectiveCompute blocks
            # the gpsimd queue for the full CC duration, which would stall
            # the branch-2 gather stream)
            xpre1 = emit_xpre(1)

            # ---- branch 2 gather/scatter (last group split into two
            # half-gathers: its matmuls gate pool stop -> xpre2 -> AllReduce)
            for g in range(ngrp2 - 1):
                emit_group(2, g)
            emit_group(2, ngrp2 - 1, split=True)

            # ---- branch-2 partial head + single merged AllReduce.
            # masif enters the output only linearly through W_out, so each
            # core pre-reduces its masked masif block to a [1, B] logit
            # contribution; payload is bf16.
            xpre2 = emit_xpre(2)
            mps = smps.tile([1, cfg.B], f32, space="PSUM", tag='spsacc')
            nc.tensor.matmul(mps[:], lhsT=woutmas_sb[:, 0, :],
                             rhs=masif_asm[1][:], start=True, stop=False)
            nc.tensor.matmul(mps[:], lhsT=woutmas_sb[:, 1, :],
                             rhs=masif_asm[2][:], start=False, stop=True)
            masc = smp.tile([1, cfg.B], bf16, tag='masc')
            nc.scalar.activation(masc[:], mps[:], AF.Identity)

            ASZ = 2 * P * cfg.B + cfg.B
            ccin = drp.tile([ASZ], bf16, tag='ccin')
            ccout = drp.tile([ASZ], bf16, tag='ccout')
            for i, t in ((0, xpre1), (1, xpre2)):
                nc.sync.dma_start(
                    out=ccin[i * P * cfg.B:(i + 1) * P * cfg.B].rearrange(
                        "(p f) -> p f", f=cfg.B),
                    in_=t[:])
            nc.sync.dma_start(
                out=ccin[2 * P * cfg.B:ASZ].rearrange(
                    "(p f) -> p f", f=cfg.B),
                in_=masc[:])
            nc.gpsimd.collective_compute(
                "AllReduce", OP.add,
                replica_groups=[list(range(N_CORES))],
                ins=[ccin[:].opt()], outs=[ccout[:].opt()])

            # ---- readback + replicated head
            xf = {}
            xf[1] = load(smp, ccout[0:P * cfg.B].rearrange(
                "(p f) -> p f", f=cfg.B), (P, cfg.B), bf16, 'x1f')
            xf[2] = load(smp, ccout[P * cfg.B:2 * P * cfg.B].rearrange(
                "(p f) -> p f", f=cfg.B), (P, cfg.B), bf16, 'x2f')
            mascf = load(smp, ccout[2 * P * cfg.B:ASZ].rearrange(
                "(p f) -> p f", f=cfg.B), (1, cfg.B), bf16, 'mascf')

            x12 = {}
            for br in (1, 2):
                xs = smp.tile([P, cfg.B], f32, tag=f'x{br}')
                nc.scalar.activation(xs[:], xf[br][:], AF.Lrelu,
                                     bias=bpf_sb[br][:, 0:1], alpha=0.01)
                x12[br] = xs

            xc1 = {}
            for mh in range(2):
                cps = smps.tile([P, cfg.B], f32, space="PSUM", tag='spsacc')
                for k2 in range(2):
                    nc.tensor.matmul(
                        cps[:], lhsT=wfc1_sb[:, k2, mh * P:(mh + 1) * P],
                        rhs=x12[k2 + 1][:], start=(k2 == 0), stop=(k2 == 1))
                xcs = smp.tile([P, cfg.B], f32, tag=f'xc{mh}')
                nc.scalar.activation(xcs[:], cps[:], AF.Lrelu,
                                     bias=bfc1_sb[:, mh, 0:1], alpha=0.01)
                xc1[mh] = xcs
            c2ps = smps.tile([64, cfg.B], f32, space="PSUM", tag='spsacc')
            for k2 in range(2):
                nc.tensor.matmul(c2ps[:], lhsT=wfc2_sb[:, k2, :],
                                 rhs=xc1[k2][:], start=(k2 == 0), stop=(k2 == 1))
            xc = smp.tile([64, cfg.B], f32, tag='xcf')
            nc.scalar.activation(xc[:], c2ps[:], AF.Lrelu,
                                 bias=bfc2_sb[:, 0:1], alpha=0.01)

            ops = smps.tile([1, cfg.B], f32, space="PSUM", tag='spsacc')
            nc.tensor.matmul(ops[:], lhsT=woutxc_sb[:, :], rhs=xc[:],
                             start=True, stop=True)
            rpre = smp.tile([1, cfg.B], f32, tag='rpre')
            nc.vector.tensor_add(out=rpre[:], in0=ops[:], in1=mascf[:])
            res = smp.tile([1, cfg.B], f32, tag='res')
            nc.scalar.activation(res[:], rpre[:], AF.Sigmoid,
                                 bias=bout_sb[:, 0:1])
            nc.sync.dma_start(out=out_t[:, :], in_=res[:])

    nc.compile()
    return nc


# ---------------------------------------------------------------- entry
_CACHE = {}


def _dts():
    import ml_dtypes
    f8 = ml_dtypes.float8_e4m3
    bf16 = ml_dtypes.bfloat16
    return {'mm': f8 if MM_FP8 else bf16,
            'xw': f8 if XW_FP8 else bf16,
            'sm': f8 if SM_FP8 else bf16}


def _run(inputs, cfg, trace=False, tmpdir=None):
    from concourse import bass_utils
    dts = _dts()
    meta, in_maps = _preprocess(inputs, cfg, dts)
    key = (cfg.N, cfg.F, meta['cpad1'], meta['cpad2'],
           tuple(x[0] for x in meta['sched1']),
           tuple(x[0] for x in meta['sched2']),
           MM_FP8, XW_FP8, SM_FP8,
           tuple(np.dtype(dts[k]).name for k in ('mm', 'xw', 'sm')))
    if key not in _CACHE:
        _CACHE.clear()
        _CACHE[key] = _build(cfg, meta, dts)
    nc = _CACHE[key]
    res = bass_utils.run_bass_kernel_spmd(
        nc, in_maps, core_ids=list(range(N_CORES)), trace=trace, tmpdir=tmpdir)
    out = np.asarray(res.results[0]['out'], np.float32).reshape(cfg.B, 1)
    return out, res


def kernel(**inputs) -> np.ndarray:
    cfg = _Cfg()
    out, _ = _run(inputs, cfg)
    return out
